# revision 24
# baseline (speedup 1.0000x reference)
"""Trainium2 Bass kernel for per-pixel kernel application (KPN-style ApplyKernel).

y[c,h,w] = sum_{ii,jj} xpad[c, h+ii, w+jj] * k[ii*11+jj, h, w]

Strategy (8 NeuronCores, data-parallel over H strips of 90 rows):
  - Partition p owns a 10-column block of W (128 partitions x 10 = 1280), with
    the +-5 column halo stored in the free dim, so both row and column shifts
    of a tap are plain access-pattern offsets (DVE lanes are partition-locked,
    so shifts must live in the free dim). All 128 lanes are used.
  - Host: pad x and build bf16 slabs [128, 3ch x 100rows x 20cols] in two
    column-alignment variants so every tap's VectorE read stays 4-byte
    aligned, keeping tensor_tensor in its 2x bf16 mode. k is re-laid-out
    host-side to bf16 [128, 121, 900] (partition-block-major, even-column
    taps first) halving HBM traffic vs f32.
  - Taps are processed in 23 runs: for each row-shift ii, the 6 even-jj (or
    5 odd-jj) taps form one run (the first ii split 2+4 so the fill-phase k
    DMA is small). Per run and channel, ONE VectorE tensor_tensor with a
    3-free-dim AP [p, tap(stride 2), row, col] computes all taps' products
    at once (bf16 2x mode, ~80ns instruction overhead amortized 6x). The
    final run falls back to per-tap ops so the PE tail stays short.
  - Products accumulate via 6 TensorE identity-matmuls per tap (K=M=128)
    into 6 PSUM banks (3 channels x 512/388-col chunks).
  - k runs are prefetched two ahead on the SP HWDGE ring; the second/third
    slab channels, ident, and the odd-alignment slab fill via the ACT ring
    in parallel.  Output y is written bf16 (host casts to f32).

  Engine budget per core (measured): DVE multiply stream ~176us (the
  bottleneck; bf16 2x mode is its ceiling), PE accumulate ~151us, DMA
  ~90us (27.9MB bf16 k + 3MB slabs + 0.7MB y at ~358GB/s/core).  GpSimd
  tensor_tensor offload was tried and rejected: a running Pool op slows
  concurrent DVE tensor_tensors ~4x (shared SBUF path), a net loss.
"""

import sys

if "/opt/trn_rl_repo" not in sys.path:
    sys.path.insert(0, "/opt/trn_rl_repo")

import numpy as np
import ml_dtypes

import concourse.mybir as mybir
from concourse import bacc
from concourse.tile import TileContext
from concourse.ap import AP
from concourse.bass_utils import run_bass_kernel_spmd

KS = 11
HALF = 5
H, W, C = 720, 1280, 3
NCORES = 8
HS = H // NCORES            # 90 rows per core
NP = 128                    # partitions (one 10-col block each)
CPP = W // NP               # 10 output cols per partition
ROWS_ST = HS + 2 * HALF     # 100 rows stored per partition
COLS_ST = CPP + 2 * HALF    # 20 cols stored per partition
SLABF = C * ROWS_ST * COLS_ST   # 6000 bf16 per partition per variant
NTAPS = KS * KS             # 121
FD = HS * CPP               # 900 elements per channel per tap
PFD = C * FD                # 2700 product elements per tap
N0, N1 = 512, FD - 512      # matmul chunk widths per channel (512, 388)

# tap order: even-jj taps first (only need slab variant 0), then odd-jj;
# within each half, ii-major so each run of same-ii taps is contiguous
TAP_PERM = ([t for t in range(NTAPS) if (t % KS) % 2 == 0]
            + [t for t in range(NTAPS) if (t % KS) % 2 == 1])
# runs of (start, ntaps): first run split 3+3 so the fill-phase k DMA is
# small, then 10 x 6 even-jj, then 11 x 5 odd-jj
RUNS = ([(0, 2), (2, 2), (4, 2)] + [(6 * i, 6) for i in range(1, 11)]
        + [(66 + 5 * i, 5) for i in range(10)] + [(116, 4), (120, 1)])
CF = ROWS_ST * COLS_ST      # 2000 elements per channel slab chunk

BF16 = ml_dtypes.bfloat16

_CACHE = {}


def _build_nc(taps=NTAPS):
    assert taps == NTAPS
    nc = bacc.Bacc("TRN2", target_bir_lowering=False, debug=False)
    k_d = nc.dram_tensor("k", [NP, NTAPS, FD], mybir.dt.bfloat16, kind="ExternalInput")
    xs_d = nc.dram_tensor("xs", [2, NP, SLABF], mybir.dt.bfloat16, kind="ExternalInput")
    id_d = nc.dram_tensor("ident", [NP, NP], mybir.dt.bfloat16, kind="ExternalInput")
    y_d = nc.dram_tensor("y", [NP, PFD], mybir.dt.bfloat16, kind="ExternalOutput")

    with TileContext(nc) as tc:
        with tc.tile_pool(name="const", bufs=1) as const_pool, \
             tc.tile_pool(name="kbf", bufs=4) as kb_pool, \
             tc.tile_pool(name="prod", bufs=2) as prod_pool, \
             tc.tile_pool(name="out", bufs=1) as out_pool, \
             tc.tile_pool(name="psum", bufs=1, space="PSUM") as psum_pool:

            # per-(variant, channel) slab tiles so the first multiply only
            # waits on its own channel's DMA
            slabs = [[const_pool.tile([NP, CF], mybir.dt.bfloat16,
                                      name=f"slab{v}c{c}")
                      for c in range(C)] for v in range(2)]
            ident = const_pool.tile([NP, NP], mybir.dt.bfloat16)

            kb_tiles = {}

            def dma_krun(r, eng):
                gi0, nt = RUNS[r]
                kb = kb_pool.tile([NP, nt * FD], mybir.dt.bfloat16, name="kb")
                eng.dma_start(
                    kb[:].rearrange("p (t f) -> p t f", t=nt),
                    k_d.ap()[:, gi0:gi0 + nt, :])
                kb_tiles[r] = kb

            # Fill: the critical path (ident, slab0-c0, k run 0) goes first
            # on the SP ring (it starts ~2.5us earlier than ACT); slab1 on
            # the ACT ring in parallel.
            xsv = xs_d.ap().rearrange("v p (c f) -> v p c f", c=C)
            dma_krun(0, nc.sync)
            nc.sync.dma_start(slabs[0][0][:], xsv[0, :, 0])
            dma_krun(1, nc.sync)
            dma_krun(2, nc.sync)
            nc.scalar.dma_start(slabs[0][1][:], xsv[0, :, 1])
            nc.scalar.dma_start(slabs[0][2][:], xsv[0, :, 2])
            nc.scalar.dma_start(ident[:], id_d.ap())

            slab_views = [
                [slabs[v][c][:].rearrange("p (r w) -> p r w", r=ROWS_ST)
                 for c in range(C)] for v in range(2)]

            accs = []
            for c in range(C):
                a0 = psum_pool.tile([NP, N0], mybir.dt.float32, name=f"acc{c}0")
                a1 = psum_pool.tile([NP, N1], mybir.dt.float32, name=f"acc{c}1")
                accs.append((a0, a1))

            nrun = len(RUNS)
            for r in range(nrun):
                gi0, nt = RUNS[r]
                if r + 2 < nrun and r >= 1:
                    dma_krun(r + 2, nc.sync)
                if r == 2:
                    # odd-alignment slab (first needed at run 12, ~95us in):
                    # deferred past the fill crunch so it doesn't steal
                    # bandwidth from the critical-path k/slab0 DMAs
                    for c in range(C):
                        nc.scalar.dma_start(slabs[1][c][:], xsv[1, :, c])
                kb = kb_tiles[r]
                t0 = TAP_PERM[gi0]
                ii = t0 // KS
                v = (t0 % KS) & 1

                prod = prod_pool.tile([NP, nt * PFD], mybir.dt.bfloat16,
                                      name="prod")
                prod_t = prod[:].rearrange("p (t f) -> p t f", t=nt)

                last_run = (r == nrun - 1)
                if not last_run:
                    # one 3-free-dim TT per channel covering all nt taps:
                    # x: [p, t(stride 2 cols), r, w]; k: [p, t(stride FD), r, w]
                    jb = (TAP_PERM[gi0] % KS) - v   # first tap's jj2
                    for c in range(C):
                        base = slab_views[v][c][:, ii:ii + HS, jb:jb + CPP]
                        xs_op = AP(base.tensor, base.offset,
                                   [list(base.ap[0])] + [[2, nt]]
                                   + [list(x) for x in base.ap[1:]])
                        kap = kb[:]
                        k_op = AP(kap.tensor, kap.offset,
                                  [list(kap.ap[0])]
                                  + [[FD, nt], [CPP, HS], [1, CPP]])
                        po = prod[:].rearrange(
                            "p (t c r w) -> p t c r w", t=nt, c=C, r=HS, w=CPP
                        )[:, :, c]
                        nc.vector.tensor_tensor(po, xs_op, k_op,
                                                mybir.AluOpType.mult)
                else:
                    # per-tap, per-channel ops so the final PE tail is short
                    for dt_ in range(nt):
                        t = TAP_PERM[gi0 + dt_]
                        jj2 = (t % KS) - v
                        for c in range(C):
                            xs_op = slab_views[v][c][:, ii:ii + HS,
                                                     jj2:jj2 + CPP]
                            k_op = (kb[:, dt_ * FD:(dt_ + 1) * FD]
                                    .rearrange("p (r w) -> p r w", r=HS))
                            po = prod_t[:, dt_].rearrange(
                                "p (c r w) -> p c r w", c=C, r=HS, w=CPP
                            )[:, c]
                            nc.vector.tensor_tensor(po, xs_op, k_op,
                                                    mybir.AluOpType.mult)

                for dt_ in range(nt):
                    gi = gi0 + dt_
                    first = (gi == 0)
                    last = (gi == taps - 1)
                    pt = prod_t[:, dt_]
                    for c in range(C):
                        nc.tensor.matmul(accs[c][0][:], ident[:],
                                         pt[:, c * FD:c * FD + N0],
                                         start=first, stop=last)
                        nc.tensor.matmul(accs[c][1][:], ident[:],
                                         pt[:, c * FD + N0:(c + 1) * FD],
                                         start=first, stop=last)

            yst = out_pool.tile([NP, PFD], mybir.dt.bfloat16)
            for c in range(C):
                nc.scalar.copy(yst[:, c * FD:c * FD + N0], accs[c][0][:])
                nc.sync.dma_start(y_d.ap()[:, c * FD:c * FD + N0],
                                  yst[:, c * FD:c * FD + N0])
                nc.vector.tensor_copy(yst[:, c * FD + N0:(c + 1) * FD],
                                      accs[c][1][:])
                nc.scalar.dma_start(y_d.ap()[:, c * FD + N0:(c + 1) * FD],
                                    yst[:, c * FD + N0:(c + 1) * FD])

    nc.compile()
    return nc


def get_nc(taps=NTAPS):
    if taps not in _CACHE:
        _CACHE[taps] = _build_nc(taps)
    return _CACHE[taps]


def _prep_inputs(x, k, padding, padding_value):
    """Host-side prep: pad x, build bf16 slabs + per-core shards."""
    x = np.asarray(x, dtype=np.float32)
    k = np.asarray(k, dtype=np.float32)
    pad = bool(int(np.asarray(padding)))
    pv = float(np.asarray(padding_value))

    if pad:
        assert x.shape == (1, C, H, W), x.shape
        xp = np.full((C, H + 2 * HALF, W + 2 * HALF + 1), 0.0, dtype=np.float32)
        xp[:, :, :W + 2 * HALF] = pv
        xp[:, HALF:HALF + H, HALF:HALF + W] = x[0]
    else:
        assert x.shape == (1, C, H + 2 * HALF, W + 2 * HALF), x.shape
        xp = np.zeros((C, H + 2 * HALF, W + 2 * HALF + 1), dtype=np.float32)
        xp[:, :, :W + 2 * HALF] = x[0]

    assert k.shape == (1, NTAPS, H, W), k.shape
    # partition-block-major, tap-permuted k: [core, p, t, (r w)], bf16
    kt_all = np.ascontiguousarray(
        k[0][TAP_PERM].astype(BF16).reshape(NTAPS, NCORES, HS, NP, CPP)
        .transpose(1, 3, 0, 2, 4)).reshape(NCORES, NP, NTAPS, FD)

    cols_idx = CPP * np.arange(NP)[:, None] + np.arange(COLS_ST)[None, :]
    ident = np.eye(NP, dtype=BF16)
    in_maps = []
    for ci in range(NCORES):
        rows = slice(HS * ci, HS * ci + ROWS_ST)
        xs = np.empty((2, NP, SLABF), dtype=BF16)
        for v in (0, 1):
            sv = xp[:, rows, v:v + W + 2 * HALF]           # [C, 100, 1290]
            win = sv[:, :, cols_idx]                       # [C, 100, 128, 20]
            xs[v] = win.transpose(2, 0, 1, 3).reshape(NP, SLABF).astype(BF16)
        in_maps.append({"k": kt_all[ci], "xs": xs, "ident": ident})
    return in_maps


def _assemble_y(results):
    """results[ci]["y"] is [128, 2700] bf16; reassemble to [1, C, H, W] f32."""
    y = np.empty((C, H, W), dtype=np.float32)
    for ci in range(NCORES):
        blk = np.asarray(results[ci]["y"], dtype=np.float32)
        blk = blk.reshape(NP, C, HS, CPP)                  # [p, c, r, w]
        y[:, HS * ci:HS * (ci + 1), :] = (
            blk.transpose(1, 2, 0, 3).reshape(C, HS, W))
    return y[None]


def kernel(x, k, padding, padding_value):
    in_maps = _prep_inputs(x, k, padding, padding_value)
    nc = get_nc()
    res = run_bass_kernel_spmd(nc, in_maps, core_ids=list(range(NCORES)))
    return _assemble_y(res.results).astype(np.float32)
